# revision 45
# baseline (speedup 1.0000x reference)
"""Multi-head attention (B=2, T=2048, C=1024, H=16) on 8 trn2 cores.

Sharding: core c -> batch b = c//4, head-group g = c%4 (4 heads, proj cols
[g*256, (g+1)*256)).  Host pre-transposes per-batch inputs to feature-major
[C, T] so every device matmul has its contraction dim on SBUF partitions.
Each core computes a partial output  O_g @ Wo[g-rows]  [2048, 1024]; the
host sums the 4 partials per batch and adds bo.

Per-core pipeline (PE-bound; ACT paces the attention inner loop):
 - attention O is packed per head-PAIR on 128 partitions so the out-proj
   contracts 128 rows per matmul (half the out-proj matmuls);
 - Q-projections for q-chunks 1-3 are deferred into the ACT-paced
   attention windows via a filler work-queue (only K/V must precede the
   first c_iter, which walks all key chunks);
 - input DMAs are issued in consumption order with the first weight/x
   transfers split in halves so the PE starts ~10us earlier;
 - the last c_iter normalizes per 128-token slice and the trailing
   out-proj units chase it, with their PSUM->SBUF copies on the by-then
   idle ACT engine.
"""

import ml_dtypes
import numpy as np


import concourse.bass as bass
import concourse.tile as tile
from concourse import bacc, mybir
from concourse.bass_utils import run_bass_kernel_spmd

B, T, C, H, D = 2, 2048, 1024, 16, 64
N_CORES = 8
GROUPS = 4          # head-groups (cores per batch)
HG = H // GROUPS    # heads per core = 4
CG = HG * D         # proj cols per core = 256
KT = C // 128       # contraction k-tiles = 8
SCALE = D ** -0.5   # 1/8

F32 = mybir.dt.float32
F32R = mybir.dt.float32r
BF16 = mybir.dt.bfloat16
AF = mybir.ActivationFunctionType



def build_mha_program():
    """Build the SPMD Bass program (identical on all 8 cores)."""
    nc = bacc.Bacc("TRN2", target_bir_lowering=False, debug=False,
                   num_devices=N_CORES)

    xqT = nc.dram_tensor("xqT", (C, T), BF16, kind="ExternalInput").ap()
    xkT = nc.dram_tensor("xkT", (C, T), BF16, kind="ExternalInput").ap()
    xvT = nc.dram_tensor("xvT", (C, T), BF16, kind="ExternalInput").ap()
    wq = nc.dram_tensor("wq", (C, CG), BF16, kind="ExternalInput").ap()
    wk = nc.dram_tensor("wk", (C, CG), BF16, kind="ExternalInput").ap()
    wv = nc.dram_tensor("wv", (C, CG), BF16, kind="ExternalInput").ap()
    bq = nc.dram_tensor("bq", (CG,), F32, kind="ExternalInput").ap()
    bk = nc.dram_tensor("bk", (CG,), F32, kind="ExternalInput").ap()
    bv = nc.dram_tensor("bv", (CG,), F32, kind="ExternalInput").ap()
    wo = nc.dram_tensor("wo", (CG, C), BF16, kind="ExternalInput").ap()
    yp = nc.dram_tensor("yp", (T, C), F32, kind="ExternalOutput").ap()

    with tile.TileContext(nc) as tc:
        _emit(tc, xqT, xkT, xvT, wq, wk, wv, bq, bk, bv, wo, yp)
    nc.compile()
    return nc


def _emit(tc, xqT, xkT, xvT, wq, wk, wv, bq, bk, bv, wo, yp):
    nc = tc.nc
    MT = CG // 128            # stationary tiles per projection = 2
    MC = 512                  # chunk width (tokens) everywhere
    NMC = T // MC             # 4 chunks
    TT = T // 128             # 16 t-tiles
    QC = 512                  # q-chunk width in attention
    NQC = T // QC             # 4 q-chunks
    VS = D + 1                # 65: V cols + ones col per head

    from contextlib import ExitStack
    with ExitStack() as ctx:
        consts = ctx.enter_context(tc.tile_pool(name="consts", bufs=1))
        xs_pool = ctx.enter_context(tc.tile_pool(name="xs", bufs=8))
        big = ctx.enter_context(tc.tile_pool(name="big", bufs=1))
        e_pool = ctx.enter_context(tc.tile_pool(name="e", bufs=6))
        ev_pool = ctx.enter_context(tc.tile_pool(name="ev", bufs=3))
        nrm_pool = ctx.enter_context(tc.tile_pool(name="nrm", bufs=4))
        pp = ctx.enter_context(tc.tile_pool(name="pp", bufs=2, space="PSUM"))
        pv_ps = ctx.enter_context(tc.tile_pool(name="pvps", bufs=2, space="PSUM"))
        sa_ps = ctx.enter_context(tc.tile_pool(name="saps", bufs=1, space="PSUM"))
        sb_ps = ctx.enter_context(tc.tile_pool(name="sbps", bufs=1, space="PSUM"))

        # Per-chunk persistent activations: fine-grained tiles so stages
        # pipeline at chunk granularity instead of a hard phase boundary.
        qTc = [big.tile([128, MT, MC], BF16, name=f"qTc{i}", tag=f"qTc{i}")
               for i in range(NMC)]
        kTc = [big.tile([128, MT, MC], BF16, name=f"kTc{i}", tag=f"kTc{i}")
               for i in range(NMC)]
        vc = [big.tile([128, MC // 128, HG * VS], BF16, name=f"vc{i}",
                       tag=f"vc{i}") for i in range(NMC)]
        # O packed by head pair: partitions 0-63 = head 2hp, 64-127 = head
        # 2hp+1, so the out-proj contracts 128 partitions per matmul.
        oc = [big.tile([128, HG // 2, QC], BF16, name=f"oc{i}", tag=f"oc{i}")
              for i in range(NQC)]

        wq_s = consts.tile([128, KT, CG], BF16, tag="wq")
        wk_s = consts.tile([128, KT, CG], BF16, tag="wk")
        wv_s = consts.tile([128, KT, CG], BF16, tag="wv")
        wo_s = consts.tile([128, HG // 2, C], BF16, tag="wo")
        bq_s = consts.tile([128, MT, 1], F32, tag="bq")
        bk_s = consts.tile([128, MT, 1], F32, tag="bk")
        bv_bc = consts.tile([128, CG], F32, tag="bv")
        ones_f = consts.tile([128, D], F32, tag="onesf")
        ones_t = consts.tile([128, D], F32R, tag="ones")

        def load_x(src, name):
            x_t = xs_pool.tile([128, KT, MC], BF16, tag="xs", name=name)
            nc.sync.dma_start(out=x_t,
                              in_=src.rearrange("(kt p) m -> p kt m", p=128))
            return x_t

        def proj_mt(x_t, w_s, b_s, dstl, mc, mt):
            ps = pp.tile([128, 512], F32, tag="pp")
            for kt in range(KT):
                nc.tensor.matmul(
                    ps[:, :MC],
                    w_s[:, kt, bass.ts(mt, 128)],
                    x_t[:, kt, :],
                    start=(kt == 0), stop=(kt == KT - 1))
            nc.vector.tensor_scalar_add(
                dstl[mc][:, mt, :], ps[:, :MC], b_s[:, mt, :])

        def emit_a(mc, xq_t, xk_t):
            for mt in range(MT):
                proj_mt(xq_t, wq_s, bq_s, qTc, mc, mt)
            for mt in range(MT):
                proj_mt(xk_t, wk_s, bk_s, kTc, mc, mt)

        def emit_ak(mc):
            xk_t = load_x(xkT[:, bass.ts(mc, MC)], f"xk{mc}")
            for mt in range(MT):
                proj_mt(xk_t, wk_s, bk_s, kTc, mc, mt)

        def emit_b(mc, xv_t=None):
            cols = bass.ts(mc, MC)
            v4 = vc[mc].rearrange("p t (h c) -> p t h c", h=HG)
            nc.vector.memset(v4[:, :, :, D:VS], 1.0)
            if xv_t is None:
                xv_t = load_x(xvT[:, cols], f"xv{mc}")
            for sub in range(MC // 128):
                ps = pp.tile([128, 512], F32, tag="pp")
                for kt in range(KT):
                    nc.tensor.matmul(
                        ps[:, :CG],
                        xv_t[:, kt, bass.ts(sub, 128)],
                        wv_s[:, kt, :],
                        start=(kt == 0), stop=(kt == KT - 1))
                nc.vector.tensor_add(
                    v4[:, sub, :, 0:D],
                    ps[:, :CG].rearrange("p (h c) -> p h c", h=HG),
                    bv_bc.rearrange("p (h c) -> p h c", h=HG))

        def emit_d_unit(qc, tl, cc, tail=False):
            trows_out = bass.ts(qc * (QC // 128) + tl, 128)
            ps = pp.tile([128, 512], F32, tag="pp")
            for hp in range(HG // 2):
                nc.tensor.matmul(
                    ps,
                    oc[qc][:, hp, bass.ts(tl, 128)],
                    wo_s[:, hp, bass.ts(cc, 512)],
                    start=(hp == 0), stop=(hp == HG // 2 - 1))
            ev = ev_pool.tile([128, 512], F32, tag="ev")
            if tail:       # ACT engine is idle once the last exp is done
                nc.scalar.copy(ev, ps)
            else:
                nc.vector.tensor_copy(ev, ps)
            nc.sync.dma_start(out=yp[trows_out, bass.ts(cc, 512)], in_=ev)

        def c_iter(qc, hp, fine=False):
            """Attention for head pair hp on q-chunk qc.  Yields after each
            4-tk block so emission can be interleaved with stage A/B."""
            po = [pv_ps.tile([128, 512], F32, tag="pv",
                             name=f"po{qc}_{hp}_{i}") for i in range(2)]
            def emit_pv(e_prev, tkp):
                for h01 in range(2):
                    nc.tensor.matmul(
                        po[h01][0:VS, :],
                        vc[tkp // 4][:, tkp % 4,
                                     bass.ds((2 * hp + h01) * VS, VS)],
                        e_prev[:, h01, :],
                        start=(tkp == 0), stop=(tkp == TT - 1))

            pending = None    # issue PV one tk late so its exp-wait is
            for tk in range(TT):   # already satisfied at the queue head
                pool = sa_ps if tk % 2 == 0 else sb_ps
                ps = pool.tile([128, 2, 512], F32,
                               tag="sa" if tk % 2 == 0 else "sb")
                for h01 in range(2):
                    pb = h01 * D
                    nc.tensor.matmul(
                        ps[:, h01, :],
                        kTc[tk // 4][pb:pb + D, hp, bass.ts(tk % 4, 128)],
                        qTc[qc][pb:pb + D, hp, :],
                        start=True, stop=True)
                e_t = e_pool.tile([128, 2, 512], BF16, tag="e")
                nc.scalar.activation(e_t, ps, AF.Exp, scale=SCALE)
                if pending is not None:
                    emit_pv(*pending)
                pending = (e_t, tk)
                if tk % 4 == 3:
                    yield
            emit_pv(*pending)
            yield
            den = nrm_pool.tile([128, 2, 512], F32R, tag="den")
            for h01 in range(2):
                nc.vector.tensor_copy(den[D:D + 1, h01, :],
                                      po[h01][D:D + 1, :])
            recs = []
            for h01 in range(2):
                rb = pp.tile([128, 512], F32, tag="pp")
                nc.tensor.matmul(rb[0:D, :],
                                 ones_t[D:D + 1, :],
                                 den[D:D + 1, h01, :],
                                 start=True, stop=True)
                rec = nrm_pool.tile([128, 512], F32, tag="rec")
                nc.vector.reciprocal_approx_fast(rec[0:D, :], rb[0:D, :])
                recs.append(rec)
            if not fine:
                for h01 in range(2):
                    nc.vector.tensor_mul(
                        oc[qc][64 * h01:64 * h01 + 64, hp, :],
                        po[h01][0:D, :], recs[h01][0:D, :])
            else:
                # final c_iter: normalize per 128-token slice and yield the
                # slice index so out-proj units can chase the normalization
                for tl in range(QC // 128):
                    sl = bass.ts(tl, 128)
                    for h01 in range(2):
                        nc.vector.tensor_mul(
                            oc[qc][64 * h01:64 * h01 + 64, hp, sl],
                            po[h01][0:D, sl], recs[h01][0:D, sl])
                    yield tl

        # ---- emission schedule (software pipeline) --------------------
        # DMA issue order = consumption order on the serial queue; the first
        # weight/x transfers are split in halves so the first matmuls start
        # as early as possible.
        xq0_t = xs_pool.tile([128, KT, MC], BF16, tag="xs", name="xq0")
        xk0_t = xs_pool.tile([128, KT, MC], BF16, tag="xs", name="xk0")
        xv0_t = xs_pool.tile([128, KT, MC], BF16, tag="xs", name="xv0")
        cols0 = bass.ts(0, MC)
        # touch Exp once so the ACT table loads during the DMA-bound head
        warm = consts.tile([128, 1], F32, tag="warm")
        nc.vector.memset(warm[0:1, :], 0.0)
        nc.scalar.activation(warm[0:1, :], warm[0:1, :], AF.Exp)
        wq_r = wq.rearrange("(kt p) c -> p kt c", p=128)
        xq0_r = xqT[:, cols0].rearrange("(kt p) m -> p kt m", p=128)
        wk_r = wk.rearrange("(kt p) c -> p kt c", p=128)
        xk0_r = xkT[:, cols0].rearrange("(kt p) m -> p kt m", p=128)
        KH = KT // 2
        nc.sync.dma_start(out=wq_s[:, 0:KH, :], in_=wq_r[:, 0:KH, :])
        nc.sync.dma_start(out=xq0_t[:, 0:KH, :], in_=xq0_r[:, 0:KH, :])
        nc.sync.dma_start(out=wq_s[:, KH:KT, :], in_=wq_r[:, KH:KT, :])
        nc.sync.dma_start(out=xq0_t[:, KH:KT, :], in_=xq0_r[:, KH:KT, :])
        nc.sync.dma_start(
            out=bq_s, in_=bq.rearrange("(mt p) -> p mt", p=128).unsqueeze(2))
        nc.sync.dma_start(out=wk_s, in_=wk_r)
        nc.sync.dma_start(out=xk0_t, in_=xk0_r)
        nc.sync.dma_start(
            out=bk_s, in_=bk.rearrange("(mt p) -> p mt", p=128).unsqueeze(2))
        nc.sync.dma_start(out=wv_s, in_=wv.rearrange("(kt p) c -> p kt c", p=128))
        nc.sync.dma_start(
            out=xv0_t, in_=xvT[:, cols0].rearrange("(kt p) m -> p kt m", p=128))
        nc.sync.dma_start(
            out=bv_bc,
            in_=bass.AP(tensor=bv.tensor, offset=bv.offset,
                        ap=[[0, 128]] + list(bv.ap)))
        nc.vector.memset(ones_f[D:D + 1, :], 1.0)
        nc.vector.tensor_copy(ones_t[D:D + 1, :], ones_f[D:D + 1, :])

        emit_a(0, xq0_t, xk0_t)        # chunk 0: Q and K (qc0 needs Q now)
        emit_b(0, xv0_t)
        g00 = c_iter(0, 0)
        next(g00)                      # tk 0-3 (chunk 0 data)
        for mc in range(1, NMC):
            emit_ak(mc)                # K+V only: Q-proj of chunks 1-3 is
            emit_b(mc)                 # deferred into later ACT-paced slots
            if mc == 1:                # wo isn't needed until the first
                nc.sync.dma_start(     # out-proj unit, well after chunk DMAs
                    out=wo_s,
                    in_=wo.rearrange("(pr p) c -> p pr c", p=128))
            next(g00, None)            # tk blocks as chunks land
        for _ in g00:                  # exhaust (norm)
            pass

        # unified filler queue: ('aq', qc, fn) deferred Q-proj pieces and
        # ('d', qc, fn) out-proj units; one item per pipeline slot.
        work_q = []
        prev_gen = None
        for qc in range(NQC):
            for hp in range(HG // 2):
                if qc == 0 and hp == 0:
                    continue
                if hp == 0 and qc > 0:
                    # scores of (qc, 0) read qTc[qc]: flush pending Q-proj
                    rest = []
                    for kind, wqc, fn in work_q:
                        if kind == 'aq' and wqc == qc:
                            fn()
                        else:
                            rest.append((kind, wqc, fn))
                    work_q = rest
                gen = c_iter(qc, hp,
                             fine=(qc == NQC - 1 and hp == HG // 2 - 1))
                next(gen)                    # prologue: tk 0-3
                if prev_gen is not None:
                    for _ in prev_gen:       # previous iteration's norm
                        pass
                    prev_gen = None
                    if hp == 0 and qc > 0:   # qc-1 fully normalized now
                        work_q += [
                            ('d', qc - 1,
                             (lambda q_, t_, c_:
                              lambda: emit_d_unit(q_, t_, c_))(qc - 1, tl, cc))
                            for tl in range(4) for cc in range(2)]
                if hp == 1:
                    # deferred Q-proj: qc0-hp1 has otherwise-idle filler
                    # slots (qc0 units aren't ready yet), so give it TWO
                    # chunks' worth; DMA now, matmuls at the queue front.
                    blk = []
                    for mcn in ([1, 2] if qc == 0 else
                                ([3] if qc == 1 else [])):
                        xq_t = load_x(xqT[:, bass.ts(mcn, MC)], f"xq{mcn}")
                        blk += [
                            ('aq', mcn,
                             (lambda x_, m_, t_:
                              lambda: proj_mt(x_, wq_s, bq_s, qTc, m_, t_))(
                                  xq_t, mcn, mt))
                            for mt in range(MT)]
                    work_q = blk + work_q
                for _ in range(2):           # boundary slot: the previous
                    if work_q:               # norm chain leaves a bigger
                        work_q.pop(0)[2]()   # PE bubble - two fillers
                for _ in range(3):           # tk 4-15
                    next(gen)
                    if work_q:
                        work_q.pop(0)[2]()
                prev_gen = gen
        for kind, wqc, fn in work_q:   # leftover fillers (ready long ago)
            fn()
        for ev in prev_gen:            # final norm: emit units per tl-slice
            if ev is not None:
                emit_d_unit(NQC - 1, ev, 0, tail=True)
                emit_d_unit(NQC - 1, ev, 1, tail=True)


_NC_CACHE = None


def _get_program():
    global _NC_CACHE
    if _NC_CACHE is None:
        _NC_CACHE = build_mha_program()
    return _NC_CACHE


def make_in_maps(query, key, value, Wq, bq, Wk, bk, Wv, bv, Wo):
    q = np.asarray(query, np.float32).reshape(B, T, C)
    k = np.asarray(key, np.float32).reshape(B, T, C)
    v = np.asarray(value, np.float32).reshape(B, T, C)
    xT = {n: [np.ascontiguousarray(a[b].T).astype(ml_dtypes.bfloat16)
              for b in range(B)]
          for n, a in (("q", q), ("k", k), ("v", v))}
    in_maps = []
    for c in range(N_CORES):
        b, g = divmod(c, GROUPS)
        sl = slice(g * CG, (g + 1) * CG)
        in_maps.append({
            "xqT": xT["q"][b], "xkT": xT["k"][b], "xvT": xT["v"][b],
            "wq": np.ascontiguousarray(np.asarray(Wq, np.float32)[:, sl]).astype(ml_dtypes.bfloat16),
            "wk": np.ascontiguousarray(np.asarray(Wk, np.float32)[:, sl]).astype(ml_dtypes.bfloat16),
            "wv": np.ascontiguousarray(np.asarray(Wv, np.float32)[:, sl]).astype(ml_dtypes.bfloat16),
            "bq": np.ascontiguousarray(np.asarray(bq, np.float32)[sl]),
            "bk": np.ascontiguousarray(np.asarray(bk, np.float32)[sl]),
            "bv": np.ascontiguousarray(np.asarray(bv, np.float32)[sl]),
            "wo": np.ascontiguousarray(np.asarray(Wo, np.float32)[sl, :]).astype(ml_dtypes.bfloat16),
        })
    return in_maps


def assemble_output(results, bo):
    y = np.zeros((B, T, C), np.float32)
    for c, res in enumerate(results):
        y[c // GROUPS] += res["yp"]
    y += np.asarray(bo, np.float32)
    return y


def kernel(query, key, value, Wq, bq, Wk, bk, Wv, bv, Wo, bo):
    nc = _get_program()
    in_maps = make_in_maps(query, key, value, Wq, bq, Wk, bk, Wv, bv, Wo)
    res = run_bass_kernel_spmd(nc, in_maps, list(range(N_CORES)))
    return assemble_output(res.results, bo)



# revision 46
# speedup vs baseline: 1.2180x; 1.2180x over previous
"""Multi-head attention (B=2, T=2048, C=1024, H=16) on 8 trn2 cores.

Sharding: core c -> batch b = c//4, head-group g = c%4 (4 heads, proj cols
[g*256, (g+1)*256)).  Host pre-transposes per-batch inputs to feature-major
[C, T] so every device matmul has its contraction dim on SBUF partitions.
Each core computes a partial output  O_g @ Wo[g-rows]  [2048, 1024]; the
host sums the 4 partials per batch and adds bo.

Per-core pipeline (PE-bound; ACT paces the attention inner loop):
 - attention O is packed per head-PAIR on 128 partitions so the out-proj
   contracts 128 rows per matmul (half the out-proj matmuls);
 - Q-projections for q-chunks 1-3 are deferred into the ACT-paced
   attention windows via a filler work-queue (only K/V must precede the
   first c_iter, which walks all key chunks);
 - input DMAs are issued in consumption order with the first weight/x
   transfers split in halves so the PE starts ~10us earlier;
 - the last c_iter normalizes per 128-token slice and the trailing
   out-proj units chase it, with their PSUM->SBUF copies on the by-then
   idle ACT engine.
"""

import ml_dtypes
import numpy as np


import concourse.bass as bass
import concourse.tile as tile
from concourse import bacc, mybir
from concourse.bass_utils import run_bass_kernel_spmd

B, T, C, H, D = 2, 2048, 1024, 16, 64
N_CORES = 8
GROUPS = 4          # head-groups (cores per batch)
HG = H // GROUPS    # heads per core = 4
CG = HG * D         # proj cols per core = 256
KT = C // 128       # contraction k-tiles = 8
SCALE = D ** -0.5   # 1/8

F32 = mybir.dt.float32
F32R = mybir.dt.float32r
BF16 = mybir.dt.bfloat16
AF = mybir.ActivationFunctionType



def build_mha_program():
    """Build the SPMD Bass program (identical on all 8 cores)."""
    nc = bacc.Bacc("TRN2", target_bir_lowering=False, debug=False,
                   num_devices=N_CORES)

    xqT = nc.dram_tensor("xqT", (C, T), BF16, kind="ExternalInput").ap()
    xkT = nc.dram_tensor("xkT", (C, T), BF16, kind="ExternalInput").ap()
    xvT = nc.dram_tensor("xvT", (C, T), BF16, kind="ExternalInput").ap()
    wq = nc.dram_tensor("wq", (C, CG), BF16, kind="ExternalInput").ap()
    wk = nc.dram_tensor("wk", (C, CG), BF16, kind="ExternalInput").ap()
    wv = nc.dram_tensor("wv", (C, CG), BF16, kind="ExternalInput").ap()
    bq = nc.dram_tensor("bq", (CG,), F32, kind="ExternalInput").ap()
    bk = nc.dram_tensor("bk", (CG,), F32, kind="ExternalInput").ap()
    bv = nc.dram_tensor("bv", (CG,), F32, kind="ExternalInput").ap()
    wo = nc.dram_tensor("wo", (CG, C), BF16, kind="ExternalInput").ap()
    yp = nc.dram_tensor("yp", (T, C), F32, kind="ExternalOutput").ap()

    with tile.TileContext(nc) as tc:
        _emit(tc, xqT, xkT, xvT, wq, wk, wv, bq, bk, bv, wo, yp)
    nc.compile()
    return nc


def _emit(tc, xqT, xkT, xvT, wq, wk, wv, bq, bk, bv, wo, yp):
    nc = tc.nc
    MT = CG // 128            # stationary tiles per projection = 2
    MC = 512                  # chunk width (tokens) everywhere
    NMC = T // MC             # 4 chunks
    TT = T // 128             # 16 t-tiles
    QC = 512                  # q-chunk width in attention
    NQC = T // QC             # 4 q-chunks
    VS = D + 1                # 65: V cols + ones col per head

    from contextlib import ExitStack
    with ExitStack() as ctx:
        consts = ctx.enter_context(tc.tile_pool(name="consts", bufs=1))
        xs_pool = ctx.enter_context(tc.tile_pool(name="xs", bufs=8))
        big = ctx.enter_context(tc.tile_pool(name="big", bufs=1))
        e_pool = ctx.enter_context(tc.tile_pool(name="e", bufs=6))
        ev_pool = ctx.enter_context(tc.tile_pool(name="ev", bufs=3))
        nrm_pool = ctx.enter_context(tc.tile_pool(name="nrm", bufs=4))
        pp = ctx.enter_context(tc.tile_pool(name="pp", bufs=2, space="PSUM"))
        pv_ps = ctx.enter_context(tc.tile_pool(name="pvps", bufs=2, space="PSUM"))
        sa_ps = ctx.enter_context(tc.tile_pool(name="saps", bufs=1, space="PSUM"))
        sb_ps = ctx.enter_context(tc.tile_pool(name="sbps", bufs=1, space="PSUM"))

        # Per-chunk persistent activations: fine-grained tiles so stages
        # pipeline at chunk granularity instead of a hard phase boundary.
        qTc = [big.tile([128, MT, MC], BF16, name=f"qTc{i}", tag=f"qTc{i}")
               for i in range(NMC)]
        kTc = [big.tile([128, MT, MC], BF16, name=f"kTc{i}", tag=f"kTc{i}")
               for i in range(NMC)]
        vc = [big.tile([128, MC // 128, HG * VS], BF16, name=f"vc{i}",
                       tag=f"vc{i}") for i in range(NMC)]
        # O packed by head pair: partitions 0-63 = head 2hp, 64-127 = head
        # 2hp+1, so the out-proj contracts 128 partitions per matmul.
        oc = [big.tile([128, HG // 2, QC], BF16, name=f"oc{i}", tag=f"oc{i}")
              for i in range(NQC)]

        wq_s = consts.tile([128, KT, CG], BF16, tag="wq")
        wk_s = consts.tile([128, KT, CG], BF16, tag="wk")
        wv_s = consts.tile([128, KT, CG], BF16, tag="wv")
        wo_s = consts.tile([128, HG // 2, C], BF16, tag="wo")
        bq_s = consts.tile([128, MT, 1], F32, tag="bq")
        bk_s = consts.tile([128, MT, 1], F32, tag="bk")
        bv_bc = consts.tile([128, CG], F32, tag="bv")
        ones_f = consts.tile([128, D], F32, tag="onesf")
        ones_t = consts.tile([128, D], F32R, tag="ones")

        def load_x(src, name):
            x_t = xs_pool.tile([128, KT, MC], BF16, tag="xs", name=name)
            nc.sync.dma_start(out=x_t,
                              in_=src.rearrange("(kt p) m -> p kt m", p=128))
            return x_t

        def proj_mt(x_t, w_s, b_s, dstl, mc, mt):
            ps = pp.tile([128, 512], F32, tag="pp")
            for kt in range(KT):
                nc.tensor.matmul(
                    ps[:, :MC],
                    w_s[:, kt, bass.ts(mt, 128)],
                    x_t[:, kt, :],
                    start=(kt == 0), stop=(kt == KT - 1))
            nc.vector.tensor_scalar_add(
                dstl[mc][:, mt, :], ps[:, :MC], b_s[:, mt, :])

        def emit_a(mc, xq_t, xk_t):
            for mt in range(MT):
                proj_mt(xq_t, wq_s, bq_s, qTc, mc, mt)
            for mt in range(MT):
                proj_mt(xk_t, wk_s, bk_s, kTc, mc, mt)

        def emit_ak(mc):
            xk_t = load_x(xkT[:, bass.ts(mc, MC)], f"xk{mc}")
            for mt in range(MT):
                proj_mt(xk_t, wk_s, bk_s, kTc, mc, mt)

        def emit_b(mc, xv_t=None):
            cols = bass.ts(mc, MC)
            v4 = vc[mc].rearrange("p t (h c) -> p t h c", h=HG)
            nc.vector.memset(v4[:, :, :, D:VS], 1.0)
            if xv_t is None:
                xv_t = load_x(xvT[:, cols], f"xv{mc}")
            for sub in range(MC // 128):
                ps = pp.tile([128, 512], F32, tag="pp")
                for kt in range(KT):
                    nc.tensor.matmul(
                        ps[:, :CG],
                        xv_t[:, kt, bass.ts(sub, 128)],
                        wv_s[:, kt, :],
                        start=(kt == 0), stop=(kt == KT - 1))
                nc.vector.tensor_add(
                    v4[:, sub, :, 0:D],
                    ps[:, :CG].rearrange("p (h c) -> p h c", h=HG),
                    bv_bc.rearrange("p (h c) -> p h c", h=HG))

        def emit_d_unit(qc, tl, cc, tail=False):
            trows_out = bass.ts(qc * (QC // 128) + tl, 128)
            ps = pp.tile([128, 512], F32, tag="pp")
            for hp in range(HG // 2):
                nc.tensor.matmul(
                    ps,
                    oc[qc][:, hp, bass.ts(tl, 128)],
                    wo_s[:, hp, bass.ts(cc, 512)],
                    start=(hp == 0), stop=(hp == HG // 2 - 1))
            ev = ev_pool.tile([128, 512], F32, tag="ev")
            if tail:       # ACT engine is idle once the last exp is done
                nc.scalar.copy(ev, ps)
            else:
                nc.vector.tensor_copy(ev, ps)
            nc.sync.dma_start(out=yp[trows_out, bass.ts(cc, 512)], in_=ev)

        def c_iter(qc, hp, fine=False):
            """Attention for head pair hp on q-chunk qc.  Yields after each
            4-tk block so emission can be interleaved with stage A/B."""
            po = [pv_ps.tile([128, 512], F32, tag="pv",
                             name=f"po{qc}_{hp}_{i}") for i in range(2)]
            def emit_pv(e_prev, tkp):
                for h01 in range(2):
                    nc.tensor.matmul(
                        po[h01][0:VS, :],
                        vc[tkp // 4][:, tkp % 4,
                                     bass.ds((2 * hp + h01) * VS, VS)],
                        e_prev[:, h01, :],
                        start=(tkp == 0), stop=(tkp == TT - 1))

            pending = None    # issue PV one tk late so its exp-wait is
            for tk in range(TT):   # already satisfied at the queue head
                pool = sa_ps if tk % 2 == 0 else sb_ps
                ps = pool.tile([128, 2, 512], F32,
                               tag="sa" if tk % 2 == 0 else "sb")
                for h01 in range(2):
                    pb = h01 * D
                    nc.tensor.matmul(
                        ps[:, h01, :],
                        kTc[tk // 4][pb:pb + D, hp, bass.ts(tk % 4, 128)],
                        qTc[qc][pb:pb + D, hp, :],
                        start=True, stop=True)
                e_t = e_pool.tile([128, 2, 512], BF16, tag="e")
                nc.scalar.activation(e_t, ps, AF.Exp, scale=SCALE)
                if pending is not None:
                    emit_pv(*pending)
                pending = (e_t, tk)
                if tk % 4 == 3:
                    yield
            emit_pv(*pending)
            yield
            den = nrm_pool.tile([128, 2, 512], F32R, tag="den")
            for h01 in range(2):
                nc.vector.tensor_copy(den[D:D + 1, h01, :],
                                      po[h01][D:D + 1, :])
            recs = []
            for h01 in range(2):
                rb = pp.tile([128, 512], F32, tag="pp")
                nc.tensor.matmul(rb[0:D, :],
                                 ones_t[D:D + 1, :],
                                 den[D:D + 1, h01, :],
                                 start=True, stop=True)
                rec = nrm_pool.tile([128, 512], F32, tag="rec")
                nc.vector.reciprocal_approx_fast(rec[0:D, :], rb[0:D, :])
                recs.append(rec)
            if not fine:
                for h01 in range(2):
                    nc.vector.tensor_mul(
                        oc[qc][64 * h01:64 * h01 + 64, hp, :],
                        po[h01][0:D, :], recs[h01][0:D, :])
            else:
                # final c_iter: normalize per 128-token slice and yield the
                # slice index so out-proj units can chase the normalization
                for tl in range(QC // 128):
                    sl = bass.ts(tl, 128)
                    for h01 in range(2):
                        nc.vector.tensor_mul(
                            oc[qc][64 * h01:64 * h01 + 64, hp, sl],
                            po[h01][0:D, sl], recs[h01][0:D, sl])
                    yield tl

        # ---- emission schedule (software pipeline) --------------------
        # DMA issue order = consumption order on the serial queue; the first
        # weight/x transfers are split in halves so the first matmuls start
        # as early as possible.
        xq0_t = xs_pool.tile([128, KT, MC], BF16, tag="xs", name="xq0")
        xk0_t = xs_pool.tile([128, KT, MC], BF16, tag="xs", name="xk0")
        xv0_t = xs_pool.tile([128, KT, MC], BF16, tag="xs", name="xv0")
        cols0 = bass.ts(0, MC)
        # touch Exp once so the ACT table loads during the DMA-bound head
        warm = consts.tile([128, 1], F32, tag="warm")
        nc.vector.memset(warm[0:1, :], 0.0)
        nc.scalar.activation(warm[0:1, :], warm[0:1, :], AF.Exp)
        wq_r = wq.rearrange("(kt p) c -> p kt c", p=128)
        xq0_r = xqT[:, cols0].rearrange("(kt p) m -> p kt m", p=128)
        wk_r = wk.rearrange("(kt p) c -> p kt c", p=128)
        xk0_r = xkT[:, cols0].rearrange("(kt p) m -> p kt m", p=128)
        KH = KT // 2
        nc.sync.dma_start(out=wq_s[:, 0:KH, :], in_=wq_r[:, 0:KH, :])
        nc.sync.dma_start(out=xq0_t[:, 0:KH, :], in_=xq0_r[:, 0:KH, :])
        nc.sync.dma_start(out=wq_s[:, KH:KT, :], in_=wq_r[:, KH:KT, :])
        nc.sync.dma_start(out=xq0_t[:, KH:KT, :], in_=xq0_r[:, KH:KT, :])
        nc.sync.dma_start(
            out=bq_s, in_=bq.rearrange("(mt p) -> p mt", p=128).unsqueeze(2))
        nc.sync.dma_start(out=wk_s, in_=wk_r)
        nc.sync.dma_start(out=xk0_t, in_=xk0_r)
        nc.sync.dma_start(
            out=bk_s, in_=bk.rearrange("(mt p) -> p mt", p=128).unsqueeze(2))
        nc.sync.dma_start(out=wv_s, in_=wv.rearrange("(kt p) c -> p kt c", p=128))
        nc.sync.dma_start(
            out=xv0_t, in_=xvT[:, cols0].rearrange("(kt p) m -> p kt m", p=128))
        nc.sync.dma_start(
            out=bv_bc,
            in_=bass.AP(tensor=bv.tensor, offset=bv.offset,
                        ap=[[0, 128]] + list(bv.ap)))
        nc.vector.memset(ones_f[D:D + 1, :], 1.0)
        nc.vector.tensor_copy(ones_t[D:D + 1, :], ones_f[D:D + 1, :])

        emit_a(0, xq0_t, xk0_t)        # chunk 0: Q and K (qc0 needs Q now)
        emit_b(0, xv0_t)
        g00 = c_iter(0, 0)
        next(g00)                      # tk 0-3 (chunk 0 data)
        for mc in range(1, NMC):
            emit_ak(mc)                # K+V only: Q-proj of chunks 1-3 is
            emit_b(mc)                 # deferred into later ACT-paced slots
            if mc == 1:                # wo isn't needed until the first
                nc.sync.dma_start(     # out-proj unit, well after chunk DMAs
                    out=wo_s,
                    in_=wo.rearrange("(pr p) c -> p pr c", p=128))
            next(g00, None)            # tk blocks as chunks land
        for _ in g00:                  # exhaust (norm)
            pass

        # unified filler queue: ('aq', qc, fn) deferred Q-proj pieces and
        # ('d', qc, fn) out-proj units; one item per pipeline slot.
        work_q = []
        prev_gen = None
        for qc in range(NQC):
            for hp in range(HG // 2):
                if qc == 0 and hp == 0:
                    continue
                if hp == 0 and qc > 0:
                    # scores of (qc, 0) read qTc[qc]: flush pending Q-proj
                    rest = []
                    for kind, wqc, fn in work_q:
                        if kind == 'aq' and wqc == qc:
                            fn()
                        else:
                            rest.append((kind, wqc, fn))
                    work_q = rest
                gen = c_iter(qc, hp,
                             fine=(qc == NQC - 1 and hp == HG // 2 - 1))
                next(gen)                    # prologue: tk 0-3
                if prev_gen is not None:
                    for _ in prev_gen:       # previous iteration's norm
                        pass
                    prev_gen = None
                    if hp == 0 and qc > 0:   # qc-1 fully normalized now
                        work_q += [
                            ('d', qc - 1,
                             (lambda q_, t_, c_:
                              lambda: emit_d_unit(q_, t_, c_))(qc - 1, tl, cc))
                            for tl in range(4) for cc in range(2)]
                if hp == 1:
                    # deferred Q-proj: qc0-hp1 has otherwise-idle filler
                    # slots (qc0 units aren't ready yet), so give it TWO
                    # chunks' worth; DMA now, matmuls at the queue front.
                    blk = []
                    for mcn in ([1, 2] if qc == 0 else
                                ([3] if qc == 1 else [])):
                        xq_t = load_x(xqT[:, bass.ts(mcn, MC)], f"xq{mcn}")
                        blk += [
                            ('aq', mcn,
                             (lambda x_, m_, t_:
                              lambda: proj_mt(x_, wq_s, bq_s, qTc, m_, t_))(
                                  xq_t, mcn, mt))
                            for mt in range(MT)]
                    work_q = blk + work_q
                if work_q:
                    work_q.pop(0)[2]()
                for _ in range(3):           # tk 4-15
                    next(gen)
                    if work_q:
                        work_q.pop(0)[2]()
                prev_gen = gen
        for kind, wqc, fn in work_q:   # leftover fillers (ready long ago)
            fn()
        for ev in prev_gen:            # final norm: emit units per tl-slice
            if ev is not None:
                emit_d_unit(NQC - 1, ev, 0, tail=True)
                emit_d_unit(NQC - 1, ev, 1, tail=True)


_NC_CACHE = None


def _get_program():
    global _NC_CACHE
    if _NC_CACHE is None:
        _NC_CACHE = build_mha_program()
    return _NC_CACHE


def make_in_maps(query, key, value, Wq, bq, Wk, bk, Wv, bv, Wo):
    q = np.asarray(query, np.float32).reshape(B, T, C)
    k = np.asarray(key, np.float32).reshape(B, T, C)
    v = np.asarray(value, np.float32).reshape(B, T, C)
    xT = {n: [np.ascontiguousarray(a[b].T).astype(ml_dtypes.bfloat16)
              for b in range(B)]
          for n, a in (("q", q), ("k", k), ("v", v))}
    in_maps = []
    for c in range(N_CORES):
        b, g = divmod(c, GROUPS)
        sl = slice(g * CG, (g + 1) * CG)
        in_maps.append({
            "xqT": xT["q"][b], "xkT": xT["k"][b], "xvT": xT["v"][b],
            "wq": np.ascontiguousarray(np.asarray(Wq, np.float32)[:, sl]).astype(ml_dtypes.bfloat16),
            "wk": np.ascontiguousarray(np.asarray(Wk, np.float32)[:, sl]).astype(ml_dtypes.bfloat16),
            "wv": np.ascontiguousarray(np.asarray(Wv, np.float32)[:, sl]).astype(ml_dtypes.bfloat16),
            "bq": np.ascontiguousarray(np.asarray(bq, np.float32)[sl]),
            "bk": np.ascontiguousarray(np.asarray(bk, np.float32)[sl]),
            "bv": np.ascontiguousarray(np.asarray(bv, np.float32)[sl]),
            "wo": np.ascontiguousarray(np.asarray(Wo, np.float32)[sl, :]).astype(ml_dtypes.bfloat16),
        })
    return in_maps


def assemble_output(results, bo):
    y = np.zeros((B, T, C), np.float32)
    for c, res in enumerate(results):
        y[c // GROUPS] += res["yp"]
    y += np.asarray(bo, np.float32)
    return y


def kernel(query, key, value, Wq, bq, Wk, bk, Wv, bv, Wo, bo):
    nc = _get_program()
    in_maps = make_in_maps(query, key, value, Wq, bq, Wk, bk, Wv, bv, Wo)
    res = run_bass_kernel_spmd(nc, in_maps, list(range(N_CORES)))
    return assemble_output(res.results, bo)

